# revision 20
# baseline (speedup 1.0000x reference)
"""TRN2 Bass kernel for nn_Base_1348619731207 (gnn_message_passing).

Model:
  graph_out = MLP_graph(mean_pool(x, batch))            # [B, G]
  node_out[b, n] = MLP_node_n(x[b, n, :])               # per-node MLPs, [B, N]
  out = concat([graph_out, node_out], axis=1)           # [B, G + N]

Sharding (8 cores): expert-parallel over the node dim N (64 nodes/core,
per-node head weights sliced with their nodes). Each core reads exactly
1/8 of x and of the per-node weights. The mean-pool partial sums come
for free from the already-resident x tiles (DVE adds), are AllReduced
across the 8 cores (128 KB), and every core then runs the tiny graph
head on the full pooled tensor.

Matmuls run in fp32r (fp32 with an 11-bit mantissa; single-pass on the
PE vs 2-pass fp32; inputs pre-rounded on host). Node pairs share each
matmul's 256-wide moving operand so the big matmuls have free-dim 256.
"""

import numpy as np

import concourse.bass as bass
import concourse.mybir as mybir
from concourse import bacc
from concourse.bass_utils import run_bass_kernel_spmd
from concourse.tile import TileContext

F32 = mybir.dt.float32
F32R = mybir.dt.float32r
RELU = mybir.ActivationFunctionType.Relu
IDENT = mybir.ActivationFunctionType.Identity

B, N, H = 128, 512, 256          # graphs, nodes/graph, hidden
DS, D1, D2, G = 128, 256, 128, 32
NCORES = 8
NPC = N // NCORES                # 64 nodes per core
NPAIR = NPC // 2                 # 32 node pairs per core
GPC = B // NCORES                # 16 graphs per core

_CACHE = {}


def _round_fp32r(a: np.ndarray) -> np.ndarray:
    """Round fp32 to fp32r (8-bit exp, 11-bit mantissa; low 12 bits zero), RNE."""
    u = np.ascontiguousarray(a, dtype=np.float32).view(np.uint32)
    low = u & np.uint32(0xFFF)
    base = u & ~np.uint32(0xFFF)
    up = (low > 0x800) | ((low == 0x800) & (((u >> np.uint32(12)) & np.uint32(1)) == 1))
    return (base + np.where(up, np.uint32(0x1000), np.uint32(0))).view(np.float32)


def _build_nc():
    nc = bacc.Bacc("TRN2", target_bir_lowering=False, debug=False)

    # Per-core inputs (shapes identical on every core).
    xt_d = nc.dram_tensor("xt", [NPAIR, 128, 512], F32R, kind="ExternalInput")
    w1_d = nc.dram_tensor("w1", [NPAIR, 128, 1024], F32R, kind="ExternalInput")
    w2_d = nc.dram_tensor("w2", [NPAIR, 128, 512], F32R, kind="ExternalInput")
    w3_d = nc.dram_tensor("w3", [128, NPC], F32R, kind="ExternalInput")
    b1_d = nc.dram_tensor("b1", [128, 2 * NPC], F32, kind="ExternalInput")
    b2_d = nc.dram_tensor("b2", [128, NPC], F32, kind="ExternalInput")
    b3_d = nc.dram_tensor("b3", [128, NPC], F32, kind="ExternalInput")
    inv_d = nc.dram_tensor("invb", [128, 256], F32, kind="ExternalInput")
    gw1_d = nc.dram_tensor("gw1", [128, 256], F32R, kind="ExternalInput")
    gw2_d = nc.dram_tensor("gw2", [128, 128], F32R, kind="ExternalInput")
    gw3_d = nc.dram_tensor("gw3", [128, 256], F32R, kind="ExternalInput")
    gw4_d = nc.dram_tensor("gw4", [128, 256], F32R, kind="ExternalInput")
    gw5_d = nc.dram_tensor("gw5", [128, G], F32R, kind="ExternalInput")
    gb1_d = nc.dram_tensor("gb1", [128, 1], F32, kind="ExternalInput")
    gb2_d = nc.dram_tensor("gb2", [128, 1], F32, kind="ExternalInput")
    gb3_d = nc.dram_tensor("gb3", [128, 2], F32, kind="ExternalInput")
    gb4_d = nc.dram_tensor("gb4", [128, 1], F32, kind="ExternalInput")
    gb5_d = nc.dram_tensor("gb5", [G, 1], F32, kind="ExternalInput")

    nout_d = nc.dram_tensor("nout", [128, NPC], F32, kind="ExternalOutput")
    gout_d = nc.dram_tensor("gout", [G, B], F32, kind="ExternalOutput")

    # collective bounce buffers (internal DRAM; output must be Shared)
    cc_in = nc.dram_tensor("cc_in", [128, 256], F32)
    cc_out = nc.dram_tensor("cc_out", [128, 256], F32, addr_space="Shared")

    with TileContext(nc) as tc:
        with (
            tc.tile_pool(name="const", bufs=1) as cst,
            tc.tile_pool(name="stream", bufs=6) as stp,
            tc.tile_pool(name="act", bufs=3) as actp,
            tc.tile_pool(name="psA", bufs=4, space=bass.MemorySpace.PSUM) as psA,
            tc.tile_pool(name="psB", bufs=4, space=bass.MemorySpace.PSUM) as psB,
        ):
            # --- constants ---
            w3t = cst.tile([128, NPC], F32R)
            nc.sync.dma_start(w3t[:], w3_d[:])
            b1t = cst.tile([128, 2 * NPC], F32)
            nc.sync.dma_start(b1t[:], b1_d[:])
            b2t = cst.tile([128, NPC], F32)
            nc.sync.dma_start(b2t[:], b2_d[:])
            b3t = cst.tile([128, NPC], F32)
            nc.sync.dma_start(b3t[:], b3_d[:])
            invt = cst.tile([128, 256], F32)
            nc.sync.dma_start(invt[:], inv_d[:])
            gw1t = cst.tile([128, 256], F32R)
            nc.sync.dma_start(gw1t[:], gw1_d[:])
            gw2t = cst.tile([128, 128], F32R)
            nc.sync.dma_start(gw2t[:], gw2_d[:])
            gw3t = cst.tile([128, 256], F32R)
            nc.sync.dma_start(gw3t[:], gw3_d[:])
            gw4t = cst.tile([128, 256], F32R)
            nc.sync.dma_start(gw4t[:], gw4_d[:])
            gw5t = cst.tile([128, G], F32R)
            nc.sync.dma_start(gw5t[:], gw5_d[:])
            gb1t = cst.tile([128, 1], F32)
            nc.sync.dma_start(gb1t[:], gb1_d[:])
            gb2t = cst.tile([128, 1], F32)
            nc.sync.dma_start(gb2t[:], gb2_d[:])
            gb3t = cst.tile([128, 2], F32)
            nc.sync.dma_start(gb3t[:], gb3_d[:])
            gb4t = cst.tile([128, 1], F32)
            nc.sync.dma_start(gb4t[:], gb4_d[:])
            gb5t = cst.tile([G, 1], F32)
            nc.sync.dma_start(gb5t[:], gb5_d[:])
            zeros = cst.tile([128, 128], F32)
            nc.gpsimd.memset(zeros[:], 0.0)

            def relu_bias(out, in_, bias, j):
                """relu(in_ + bias): ACT for j=0, DVE for j=1 (load balance)."""
                if j == 0:
                    nc.scalar.activation(out, in_, RELU, bias=bias)
                else:
                    nc.vector.scalar_tensor_tensor(
                        out, in_, bias, zeros[:, :in_.shape[-1]],
                        mybir.AluOpType.add, mybir.AluOpType.max,
                    )

            # node_out staging in SBUF (written column-pair at a time)
            nout_sb = cst.tile([128, NPC], F32)
            # pooling partial sums over this core's nodes: [h%128, kh*128+b]
            pacc0 = cst.tile([128, 256], F32)
            nc.gpsimd.memset(pacc0[:], 0.0)
            pacc1 = cst.tile([128, 256], F32)
            nc.gpsimd.memset(pacc1[:], 0.0)
            paccs = [pacc0, pacc1]

            # --- node-pair loop ---
            for i in range(NPAIR):
                xt = stp.tile([128, 512], F32R, tag="xt")
                nc.sync.dma_start(xt[:], xt_d[i])
                w1 = stp.tile([128, 1024], F32R, tag="w1")
                nc.sync.dma_start(w1[:], w1_d[i])
                w2 = stp.tile([128, 512], F32R, tag="w2")
                nc.sync.dma_start(w2[:], w2_d[i])

                # consecutive matmuls alternate psum banks (j innermost);
                # each (j, mh) accumulation group closes before the same
                # bank's next group opens
                h1 = actp.tile([128, 512], F32R, tag="h1")
                p1a = psA.tile([128, 512], F32, tag="p1")
                p1b = psA.tile([128, 512], F32, tag="p1")
                p1s = [p1a, p1b]
                for mh in range(2):
                    for kh in range(2):
                        for j in range(2):
                            nc.tensor.matmul(
                                p1s[j][:, mh * 256:(mh + 1) * 256],
                                w1[:, ((j * 2 + kh) * 2 + mh) * 128:
                                     ((j * 2 + kh) * 2 + mh + 1) * 128],
                                xt[:, kh * 256:(kh + 1) * 256],
                                start=(kh == 0), stop=(kh == 1),
                            )
                for mh in range(2):
                    for j in range(2):
                        # keep only this node's 128 columns of the 256-wide out
                        relu_bias(
                            h1[:, mh * 256 + j * 128: mh * 256 + (j + 1) * 128],
                            p1s[j][:, mh * 256 + j * 128: mh * 256 + (j + 1) * 128],
                            b1t[:, (2 * i + j) * 2 + mh:(2 * i + j) * 2 + mh + 1],
                            j,
                        )

                p2a = psB.tile([128, 256], F32, tag="p2")
                p2b = psB.tile([128, 256], F32, tag="p2")
                p2s = [p2a, p2b]
                h2 = actp.tile([128, 256], F32R, tag="h2")
                for dh in range(2):
                    for j in range(2):
                        nc.tensor.matmul(
                            p2s[j][:],
                            w2[:, (j * 2 + dh) * 128:(j * 2 + dh + 1) * 128],
                            h1[:, dh * 256:(dh + 1) * 256],
                            start=(dh == 0), stop=(dh == 1),
                        )
                for j in range(2):
                    relu_bias(
                        h2[:, j * 128:(j + 1) * 128],
                        p2s[j][:, j * 128:(j + 1) * 128],
                        b2t[:, 2 * i + j:2 * i + j + 1],
                        j,
                    )

                # L3: fp32r needs even dst columns -> N=2 per node (one
                # garbage column each), staged per-j then combined with b3
                p3a = psB.tile([128, 2], F32, tag="p2")
                p3b = psB.tile([128, 2], F32, tag="p2")
                for j, p3 in ((0, p3a), (1, p3b)):
                    nc.tensor.matmul(
                        p3[:],
                        h2[:, j * 128:(j + 1) * 128],
                        w3t[:, 2 * i:2 * i + 2],
                        start=True, stop=True,
                    )
                for j, p3 in ((0, p3a), (1, p3b)):
                    n_loc = 2 * i + j
                    nc.vector.tensor_add(
                        nout_sb[:, n_loc:n_loc + 1],
                        p3[:, j:j + 1],
                        b3t[:, n_loc:n_loc + 1],
                    )

                # pooling partials from the same xt tile:
                # pacc_j[p, kh*128+b] += xt[p, kh*256 + j*128 + b]
                xt3 = xt[:].rearrange("p (kh jb) -> p kh jb", kh=2)
                pv0 = pacc0[:].rearrange("p (kh b) -> p kh b", kh=2)
                pv1 = pacc1[:].rearrange("p (kh b) -> p kh b", kh=2)
                nc.vector.tensor_add(pv0, pv0, xt3[:, :, 0:128].bitcast(F32))
                nc.vector.tensor_add(pv1, pv1, xt3[:, :, 128:256].bitcast(F32))

            nc.sync.dma_start(nout_d[:], nout_sb[:])

            # --- pooling: combine partials, AllReduce across the 8 cores ---
            nc.vector.tensor_add(pacc0[:], pacc0[:], pacc1[:])
            nc.sync.dma_start(cc_in[:], pacc0[:])
            nc.gpsimd.collective_compute(
                "AllReduce",
                mybir.AluOpType.add,
                replica_groups=[list(range(NCORES))],
                ins=[cc_in[:]],
                outs=[cc_out[:]],
            )
            xsum = cst.tile([128, 256], F32)
            nc.sync.dma_start(xsum[:], cc_out[:])

            # relu(sum / counts)  (counts broadcast per graph along free dim)
            xm = actp.tile([128, 256], F32, tag="ga")
            nc.vector.tensor_mul(xm[:], xsum[:], invt[:])
            xgr = actp.tile([128, 256], F32R, tag="ga2")
            nc.scalar.activation(xgr[:], xm[:], RELU)

            # --- graph head (all B graphs on every core; fp32r) ---
            # layer 1: relu(x_graph) @ gs_w1 + gs_b1   (no relu after)
            g1 = psB.tile([128, B], F32, tag="p2")
            for kh in range(2):
                nc.tensor.matmul(
                    g1[:], gw1t[:, kh * 128:(kh + 1) * 128],
                    xgr[:, kh * 128:(kh + 1) * 128],
                    start=(kh == 0), stop=(kh == 1),
                )
            a1 = actp.tile([128, B], F32R, tag="ga")
            nc.scalar.activation(a1[:], g1[:], IDENT, bias=gb1t[:, 0:1])
            # layer 2: relu(a1 @ gs_w2 + gs_b2)
            g2 = psB.tile([128, B], F32, tag="p2")
            nc.tensor.matmul(g2[:], gw2t[:], a1[:], start=True, stop=True)
            a2 = actp.tile([128, B], F32R, tag="ga2")
            nc.scalar.activation(a2[:], g2[:], RELU, bias=gb2t[:, 0:1])
            # layer 3: relu(a2 @ gh_w1 + gh_b1)  (D1 = 256 -> two M halves)
            a3 = actp.tile([128, 2 * B], F32R, tag="ga3")
            for mh in range(2):
                g3 = psB.tile([128, B], F32, tag="p2")
                nc.tensor.matmul(
                    g3[:], gw3t[:, mh * 128:(mh + 1) * 128], a2[:],
                    start=True, stop=True,
                )
                nc.scalar.activation(
                    a3[:, mh * B:(mh + 1) * B], g3[:], RELU,
                    bias=gb3t[:, mh:mh + 1],
                )
            # layer 4: relu(a3 @ gh_w2 + gh_b2)
            g4 = psB.tile([128, B], F32, tag="p2")
            for kh in range(2):
                nc.tensor.matmul(
                    g4[:], gw4t[:, kh * 128:(kh + 1) * 128],
                    a3[:, kh * B:(kh + 1) * B],
                    start=(kh == 0), stop=(kh == 1),
                )
            a4 = actp.tile([128, B], F32R, tag="ga")
            nc.scalar.activation(a4[:], g4[:], RELU, bias=gb4t[:, 0:1])
            # layer 5: a4 @ gh_w3 + gh_b3
            g5 = psB.tile([G, B], F32, tag="p2")
            nc.tensor.matmul(g5[:], gw5t[:], a4[:], start=True, stop=True)
            gout_sb = actp.tile([G, B], F32, tag="gout")
            nc.scalar.activation(gout_sb[:], g5[:], IDENT, bias=gb5t[:, 0:1])
            nc.sync.dma_start(gout_d[:], gout_sb[:])

    nc.compile()
    return nc


def _prep_core_inputs(c, xr, nh_w1, nh_b1, nh_w2, nh_b2, nh_w3, nh_b3, shared):
    ns = slice(c * NPC, (c + 1) * NPC)
    xv = xr.reshape(B, N, H)

    # xt: [pair, p(h%128), kh, j, b] -> [NPAIR, 128, 512]
    xt = (
        xv[:, ns, :]                              # [b, n, h]
        .reshape(B, NPAIR, 2, 2, 128)             # b, pair, j, kh, p
        .transpose(1, 4, 3, 2, 0)                 # pair, p, kh, j, b
        .reshape(NPAIR, 128, 512)
    )
    xt = np.ascontiguousarray(xt)

    w1 = (
        nh_w1[ns]                                 # [n, h, d1]
        .reshape(NPAIR, 2, 2, 128, 2, 128)        # pair, j, kh, p, mh, m
        .transpose(0, 3, 1, 2, 4, 5)              # pair, p, j, kh, mh, m
        .reshape(NPAIR, 128, 1024)
    )
    w1 = _round_fp32r(w1)

    w2 = (
        nh_w2[ns]                                 # [n, d1, d2]
        .reshape(NPAIR, 2, 2, 128, 128)           # pair, j, dh, p, d2
        .transpose(0, 3, 1, 2, 4)                 # pair, p, j, dh, d2
        .reshape(NPAIR, 128, 512)
    )
    w2 = _round_fp32r(w2)

    w3 = _round_fp32r(np.ascontiguousarray(nh_w3[ns, :, 0].T))      # [128, NPC]
    b1 = np.ascontiguousarray(
        nh_b1[ns].reshape(NPC, 2, 128).transpose(2, 0, 1).reshape(128, 2 * NPC)
    )
    b2 = np.ascontiguousarray(nh_b2[ns].T)                          # [128, NPC]
    b3 = np.ascontiguousarray(
        np.broadcast_to(nh_b3[ns].reshape(1, NPC), (128, NPC))
    )

    d = {"xt": xt, "w1": w1, "w2": w2, "w3": w3, "b1": b1, "b2": b2, "b3": b3}
    d.update(shared)
    return d


def kernel(x, batch, gs_w1, gs_b1, gs_w2, gs_b2,
           gh_w1, gh_b1, gh_w2, gh_b2, gh_w3, gh_b3,
           nh_w1, nh_b1, nh_w2, nh_b2, nh_w3, nh_b3):
    x = np.asarray(x, np.float32)
    batch = np.asarray(batch, np.int32)

    counts = np.bincount(batch, minlength=B).astype(np.float32)
    inv_counts = np.where(counts > 0, 1.0 / np.maximum(counts, 1), 0.0).astype(
        np.float32
    )
    uniform = bool(np.array_equal(batch, (np.arange(B * N) // N).astype(np.int32)))

    xr = _round_fp32r(x)

    # inverse counts laid out as [p, kh*128 + b] (same for every partition)
    invb = np.ascontiguousarray(
        np.broadcast_to(np.tile(inv_counts, 2)[None, :], (128, 256))
    ).astype(np.float32)

    shared = {
        "invb": invb,
        "gw1": _round_fp32r(
            np.asarray(gs_w1, np.float32).reshape(2, 128, 128)
            .transpose(1, 0, 2).reshape(128, 256)
        ),
        "gw2": _round_fp32r(np.asarray(gs_w2, np.float32)),
        "gw3": _round_fp32r(np.asarray(gh_w1, np.float32)),
        "gw4": _round_fp32r(
            np.asarray(gh_w2, np.float32).reshape(2, 128, 128)
            .transpose(1, 0, 2).reshape(128, 256)
        ),
        "gw5": _round_fp32r(np.asarray(gh_w3, np.float32)),
        "gb1": np.asarray(gs_b1, np.float32).reshape(128, 1),
        "gb2": np.asarray(gs_b2, np.float32).reshape(128, 1),
        "gb3": np.ascontiguousarray(np.asarray(gh_b1, np.float32).reshape(2, 128).T),
        "gb4": np.asarray(gh_b2, np.float32).reshape(128, 1),
        "gb5": np.asarray(gh_b3, np.float32).reshape(G, 1),
    }

    nh_w1 = np.asarray(nh_w1, np.float32)
    nh_w2 = np.asarray(nh_w2, np.float32)
    nh_w3 = np.asarray(nh_w3, np.float32)
    nh_b1 = np.asarray(nh_b1, np.float32)
    nh_b2 = np.asarray(nh_b2, np.float32)
    nh_b3 = np.asarray(nh_b3, np.float32)

    if "nc" not in _CACHE:
        _CACHE["nc"] = _build_nc()
    nc = _CACHE["nc"]

    in_maps = [
        _prep_core_inputs(c, xr, nh_w1, nh_b1, nh_w2, nh_b2, nh_w3, nh_b3,
                          shared)
        for c in range(NCORES)
    ]

    res = run_bass_kernel_spmd(nc, in_maps, core_ids=list(range(NCORES)))
    _CACHE["last_result"] = res

    out = np.empty((B, G + N), np.float32)
    out[:, :G] = res.results[0]["gout"].T
    for c in range(NCORES):
        out[:, G + NPC * c:G + NPC * (c + 1)] = res.results[c]["nout"]

    if not uniform:
        # device pooling assumes batch == arange(B*N) // N; fall back to a
        # host graph head for any other (still sorted) assignment
        xg = np.zeros((B, H), np.float64)
        np.add.at(xg, batch, x.astype(np.float64))
        xg /= np.maximum(counts, 1).astype(np.float64)[:, None]
        g = np.maximum(xg, 0) @ np.asarray(gs_w1, np.float64) + np.asarray(gs_b1)
        g = np.maximum(g @ np.asarray(gs_w2, np.float64) + np.asarray(gs_b2), 0)
        g = np.maximum(g @ np.asarray(gh_w1, np.float64) + np.asarray(gh_b1), 0)
        g = np.maximum(g @ np.asarray(gh_w2, np.float64) + np.asarray(gh_b2), 0)
        out[:, :G] = (g @ np.asarray(gh_w3, np.float64)
                      + np.asarray(gh_b3)).astype(np.float32)

    return out
